# revision 18
# baseline (speedup 1.0000x reference)
"""Trainium2 Bass kernel for DetectionLoss (focal cls + DFL box loss).

v3 strategy: DRAM-side sparse gather with latency-trimmed DMA chain.
-------------------------------------------------------------------
The loss reads the feature maps at only 50 target locations per image,
so per core the device gathers 128 padded targets x 144 channels
(576B runs) from a host-transposed channels-last copy of the feature
shard, instead of streaming the 9.7MB shard.  The per-rep critical path
is three serial DMA round trips (in -> gather -> out) plus the compute
between them; v3 trims each leg:

  1. Input DMA carries only 23 packed columns (targets + consts) on the
     SP/HWDGE queue (~2.8us incl. fixed DGE+sem latency).  iota rows are
     generated on-device by Pool while the DMA is in flight.
  2. Position math is vectorized over all 3 FPN layers at once on
     [128,6]/[128,3] tiles (floor emulated robustly: HW f32->i32 cast
     rounds-half-even, CoreSim truncates), ~11 DVE ops.
  3. One indirect SWDGE gather (128 x 576B descriptors) lands features
     in loss layout [target, channel].
  4. Both Exp and Ln come from act-func-set 6 (natural_log_exp_and_
     others), forced via a get_activation_tables patch, so the single
     1283ns table load runs at t=0 instead of mid-kernel.
  5. Loss math is spread across ACT/DVE/Pool with fused
     scalar_tensor_tensor(+accum) ops (tensor_tensor_reduce is rejected
     by this neuronxcc, so it is avoided).
  6. The 2-scalar result (valid-mask matmul in PSUM) leaves via a
     dma_scatter_add whose descriptors are PREPARED during the gather
     window and only TRIGGERED when the result tile is ready -- the
     output leg pays dge+transfer+sem (~1.6us) instead of the full
     ~2.75us HWDGE chain.  Distinct indices (0,1) make the "add" a
     plain store into the zero-initialized output buffer.

Host side does layout-only prep (transpose/concat/pad); all arithmetic
(indexing, softmax, focal, DFL, reduction) runs on device.  The 8 cores
are data-parallel over the batch; the host sums the 8 (cls, box) pairs.
"""

import numpy as np

import concourse.bass as bass
import concourse.hw_specs as hw_specs
import concourse.mybir as mybir
import concourse.tile as tile
from concourse.tile import add_dep_helper
from concourse import bacc
from concourse.bass_utils import run_bass_kernel_spmd

F32 = mybir.dt.float32
I32 = mybir.dt.int32
I16 = mybir.dt.int16
ALU = mybir.AluOpType
ACT = mybir.ActivationFunctionType
AX = mybir.AxisListType

OUT_MODE = "plain"   # "scatter" (prep+trigger) is broken on HW ucode: q0 wedges, q1 reads stale src; keep Pool dma_start

N_CORES = 8
B = 16
BPC = B // N_CORES          # images per core
N_TGT = 50
NT_PAD = 64                 # padded targets per image
NJ = BPC * NT_PAD           # 128 padded targets per core
N_CLS = 80
N_BINS = 16
C = 4 * N_BINS + N_CLS      # 144
S0, S1, S2 = 6400, 1600, 400
NPOS = BPC * (S0 + S1 + S2)            # 16800 spatial positions/core
SB0, SB1, SB2 = 0, BPC * S0, BPC * (S0 + S1)  # layer position bases

# packed-constant column layout (f32 columns)
CP_CLS = 0
CP_CX = 1
CP_CY = 2
CP_W = 3
CP_H = 4
CP_LY = 5
CP_VALID = 6
CP_IOTA3 = 7                # 0,1,2
CP_W6 = 10                  # 80,40,20,80,40,20
CP_BASE3 = 16               # SB_l + (j//64)*S_l
CP_SIDX = 19                # 8 x i16 scatter idxs [0,1,-1,...] (4 f32 cols)
CP_COLS = 23


def _patch_act_tables():
    """Force every Exp/Ln activation onto act-func-set 6
    (natural_log_exp_and_others) so one t=0 table load covers both.
    Only the set-selection input to Bacc's load-insertion pass changes;
    emitted ids still index the real act_info.json."""
    if getattr(bacc, "_act_tables_patched", False):
        return
    orig = hw_specs.get_activation_tables
    both = {ACT.Exp, ACT.Ln}

    def patched(arch):
        tabs = orig(arch)
        out = {}
        for name, funcs in tabs.items():
            if name == "natural_log_exp_and_others":
                out[name] = set(funcs)
            else:
                out[name] = set(funcs) - both
        return out

    bacc.get_activation_tables = patched
    bacc._act_tables_patched = True


def _emit(nc, tc, io, pools, mode="full"):
    pw, pp = pools

    def ts(eng, dst, src, s1, s2, o1, o2=None):
        if o2 is None:
            eng.tensor_scalar(dst, src, s1, s2, o1)
        else:
            eng.tensor_scalar(dst, src, s1, s2, o1, o2)

    V, P, A = nc.vector, nc.gpsimd, nc.scalar

    # ---- t0: SP issues the input DMA; Pool generates iota rows ----
    cp = pw.tile([128, CP_COLS], F32, tag="cp")
    nc.sync.dma_start(cp[:], io["cpack"])
    io80 = pw.tile([128, N_CLS], F32, tag="io80")
    P.iota(io80[:], pattern=[[1, N_CLS]], base=0, channel_multiplier=0,
           allow_small_or_imprecise_dtypes=True)
    io64 = pw.tile([128, 64], F32, tag="io64")
    P.iota(io64[:], pattern=[[0, 4], [1, N_BINS]], base=0,
           channel_multiplier=0, allow_small_or_imprecise_dtypes=True)

    cls_c = cp[:, CP_CLS:CP_CLS + 1]
    ly_c = cp[:, CP_LY:CP_LY + 1]
    valid = cp[:, CP_VALID:CP_VALID + 1]
    iota3 = cp[:, CP_IOTA3:CP_IOTA3 + 3]
    w6 = cp[:, CP_W6:CP_W6 + 6]
    w3 = cp[:, CP_W6:CP_W6 + 3]
    base3 = cp[:, CP_BASE3:CP_BASE3 + 3]
    sidx = cp[:, CP_SIDX:CP_SIDX + 4].bitcast(I16)

    # ---- position index per target (vectorized over 3 layers) ----
    cxy_b = cp[:, CP_CX:CP_CX + 2].unsqueeze(2).to_broadcast([128, 2, 3])
    c6 = pw.tile([128, 2, 3], F32, tag="c6")
    V.tensor_tensor(c6[:], cxy_b, w6[:].rearrange("p (a b) -> p a b", b=3),
                    ALU.mult)
    r6 = pw.tile([128, 2, 3], I32, tag="r6")
    V.tensor_copy(r6[:], c6[:])
    f6 = pw.tile([128, 2, 3], F32, tag="f6")
    V.tensor_copy(f6[:], r6[:])
    a6 = pw.tile([128, 2, 3], F32, tag="a6")
    V.tensor_tensor(a6[:], f6[:], c6[:], ALU.is_gt)
    s6 = pw.tile([128, 2, 3], F32, tag="s6")
    V.tensor_tensor(s6[:], f6[:], a6[:], ALU.subtract)  # [fx3 | fy3]
    E = pw.tile([128, 3], F32, tag="E")
    V.tensor_tensor(E[:], iota3, ly_c.to_broadcast([128, 3]), ALU.is_equal)
    m3 = pw.tile([128, 3], F32, tag="m3")
    V.tensor_tensor(m3[:], s6[:, 1, :], w3, ALU.mult)
    V.tensor_tensor(m3[:], m3[:], s6[:, 0, :], ALU.add)
    V.tensor_tensor(m3[:], m3[:], base3, ALU.add)
    pj = pw.tile([128, 3], F32, tag="pj")
    posf = pw.tile([128, 1], F32, tag="posf")
    V.scalar_tensor_tensor(pj[:], m3[:], 1.0, E[:], ALU.mult, ALU.mult,
                           accum_out=posf[:])
    ix = pw.tile([128, 1], I32, tag="ix")
    ix_inst = V.tensor_copy(ix[:], posf[:])  # exact int; cast mode free

    # ---- gather target columns from HBM (128 x 576B), then prep out ----
    T = pw.tile([128, C], F32, tag="T")
    P.indirect_dma_start(
        out=T[:], out_offset=None, in_=io["feat"],
        in_offset=bass.IndirectOffsetOnAxis(ap=ix[:], axis=0))
    if mode == "gather":
        P.dma_start(io["out"], T[0:2, 0:1])
        return

    # ---- target-only precomputes (overlap the gather) ----
    # nosync edges on ix keep these off the critical pos chain: the Tile
    # list scheduler is greedy and would otherwise slot them into the
    # chain's semaphore-wait bubbles, displacing the gather by ~800ns.
    pre = []
    oh = pw.tile([128, N_CLS], F32, tag="oh")
    pre.append(V.tensor_tensor(oh[:], io80[:],
                               cls_c.to_broadcast([128, N_CLS]),
                               ALU.is_equal))
    Wt = pw.tile([128, 1], F32, tag="Wt")
    ew = pw.tile([128, 3], F32, tag="ew")
    pre.append(V.scalar_tensor_tensor(ew[:], E[:], 1.0, w3, ALU.mult,
                                      ALU.mult, accum_out=Wt[:]))
    hh = pw.tile([128, 1], F32, tag="hh")
    pre.append(V.tensor_scalar(hh[:], Wt[:], 0.5, None, ALU.mult))
    g1 = pw.tile([128, 1], F32, tag="g1")
    g2 = pw.tile([128, 1], F32, tag="g2")
    pre.append(V.tensor_tensor(g1[:], cp[:, CP_W:CP_W + 1], hh[:],
                               ALU.mult))
    pre.append(V.tensor_tensor(g2[:], cp[:, CP_H:CP_H + 1], hh[:],
                               ALU.mult))
    t4 = pw.tile([128, 4], F32, tag="t4")
    t4v = t4[:].rearrange("p (a b) -> p a b", b=2)
    pre.append(V.tensor_copy(t4v[:, :, 0:1],
                             g1[:].unsqueeze(2).to_broadcast([128, 2, 1])))
    pre.append(V.tensor_copy(t4v[:, :, 1:2],
                             g2[:].unsqueeze(2).to_broadcast([128, 2, 1])))
    pre.append(V.tensor_scalar(t4[:], t4[:],
                               float(N_BINS - 1 - 1e-06), None, ALU.min))
    # tent weights Wb[t,(a,k)] = relu(1 - |k - t4[t,a]|)
    Wb = pw.tile([128, 64], F32, tag="Wb")
    pre.append(V.tensor_tensor(
        Wb[:].rearrange("p (a b) -> p a b", b=N_BINS),
        io64[:].rearrange("p (a b) -> p a b", b=N_BINS),
        t4[:].unsqueeze(2).to_broadcast([128, 4, N_BINS]),
        ALU.subtract))
    wa = pw.tile([128, 64], F32, tag="wa")
    pre.append(V.tensor_scalar(wa[:], Wb[:], -1.0, 1.0, ALU.mult, ALU.add))
    pre.append(V.tensor_scalar(Wb[:], Wb[:], 1.0, None, ALU.add))
    pre.append(V.tensor_tensor(Wb[:], wa[:], Wb[:], ALU.min))
    pre.append(V.tensor_scalar(Wb[:], Wb[:], 0.0, None, ALU.max))
    for p_ in pre:
        add_dep_helper(p_.ins, ix_inst.ins, sync=False,
                       reason="precompute after pos chain")

    # ---- post-gather loss math ----
    d64 = T[:, 0:64]
    z80 = T[:, 64:C]
    lt = pw.tile([128, 2], F32, tag="lt")   # [sez, prod4]

    ed = pw.tile([128, 64], F32, tag="ed")
    A.activation(ed[:], d64, ACT.Exp)
    ez = pw.tile([128, N_CLS], F32, tag="ez")
    A.activation(ez[:], z80, ACT.Exp, accum_out=lt[:, 0:1])

    wdj = pw.tile([128, 64], F32, tag="wdj")
    wds = pw.tile([128, 1], F32, tag="wds")
    V.scalar_tensor_tensor(wdj[:], Wb[:], 1.0, d64, ALU.mult, ALU.mult,
                           accum_out=wds[:])
    zmj = pw.tile([128, N_CLS], F32, tag="zmj")
    zsel = pw.tile([128, 1], F32, tag="zsel")
    V.scalar_tensor_tensor(zmj[:], z80, 1.0, oh[:], ALU.mult, ALU.mult,
                           accum_out=zsel[:])
    se4 = pw.tile([128, 4], F32, tag="se4")
    V.tensor_reduce(se4[:], ed[:].rearrange("p (a b) -> p a b", b=N_BINS),
                    AX.X, ALU.add)
    pr2 = pw.tile([128, 2], F32, tag="pr2")
    V.tensor_tensor(pr2[:], se4[:, 0:2], se4[:, 2:4], ALU.mult)
    V.tensor_tensor(lt[:, 1:2], pr2[:, 0:1], pr2[:, 1:2], ALU.mult)

    emj = pw.tile([128, N_CLS], F32, tag="emj")
    esel = pw.tile([128, 1], F32, tag="esel")
    V.scalar_tensor_tensor(emj[:], ez[:], 1.0, oh[:], ALU.mult, ALU.mult,
                           accum_out=esel[:])

    ll = pw.tile([128, 2], F32, tag="ll")   # [lse, ln prod4]
    A.activation(ll[:], lt[:], ACT.Ln)

    rse = pw.tile([128, 1], F32, tag="rse")
    V.reciprocal(rse[:], lt[:, 0:1])
    pt = pw.tile([128, 1], F32, tag="pt")
    V.tensor_tensor(pt[:], esel[:], rse[:], ALU.mult)
    u1 = pw.tile([128, 1], F32, tag="u1")
    ts(V, u1[:], pt[:], -1.0, 1.0, ALU.mult, ALU.add)
    u2 = pw.tile([128, 1], F32, tag="u2")
    V.tensor_tensor(u2[:], u1[:], u1[:], ALU.mult)
    S = pw.tile([128, 2], F32, tag="S")
    cev = pw.tile([128, 1], F32, tag="cev")
    V.scalar_tensor_tensor(cev[:], ll[:, 0:1], zsel[:], valid,
                           ALU.subtract, ALU.mult)
    V.tensor_tensor(S[:, 0:1], u2[:], cev[:], ALU.mult)
    V.scalar_tensor_tensor(S[:, 1:2], ll[:, 1:2], wds[:], valid,
                           ALU.subtract, ALU.mult)

    # ---- reduce 128 per-target contributions to 2 scalars, send out ----
    # valid is already folded into both S columns (ones column would do;
    # reusing valid is free).  gpsimd.tensor_reduce(axis=C) is a Q7
    # software op (~150us on HW!) -- use the PE matmul instead.
    PS = pp.tile([2, 1], F32, tag="PS")
    nc.tensor.matmul(PS[:], S[:], valid, start=True, stop=True)
    osb = pw.tile([2, 1], F32, tag="osb")
    V.tensor_copy(osb[:], PS[:])
    P.dma_start(io["out"], osb[:])


_CACHE = {}


def _build(reps=1, mode="full"):
    key = f"nc{reps}_{mode}_{OUT_MODE}"
    if key in _CACHE:
        return _CACHE[key], _CACHE[key + "_names"]
    _patch_act_tables()
    nc = bacc.Bacc("TRN2", target_bir_lowering=False, debug=False,
                   enable_asserts=False, num_devices=N_CORES,
                   num_swdge_queues=2)
    io = {}
    io["feat"] = nc.dram_tensor("feat", [NPOS, C], F32,
                                kind="ExternalInput").ap()
    io["cpack"] = nc.dram_tensor("cpack", [128, CP_COLS], F32,
                                 kind="ExternalInput").ap()
    io["out"] = nc.dram_tensor("out", [2, 1], F32,
                               kind="ExternalOutput").ap()

    with tile.TileContext(nc) as tc:
        with tc.tile_pool(name="wk", bufs=1) as pw, \
             tc.tile_pool(name="ps", bufs=1, space="PSUM") as pp:
            for r in range(reps):
                if r:
                    tc.strict_bb_all_engine_barrier()
                _emit(nc, tc, io, (pw, pp), mode=mode)
    nc.compile()
    _CACHE[key] = nc
    _CACHE[key + "_names"] = list(io)
    return nc, list(io)


def _const_block():
    if "cblk" in _CACHE:
        return _CACHE["cblk"]
    j = np.arange(NJ)
    sidx = np.full((128, 8), -1, np.int16)
    sidx[0, 0], sidx[1, 0] = 0, 1  # wrapped [p,s]: token j at [j%16, j//16]
    out = {
        "valid": ((j % NT_PAD) < N_TGT).astype(np.float32)[:, None],
        "iota3": np.broadcast_to(np.arange(3, dtype=np.float32),
                                 (128, 3)).copy(),
        "w6": np.broadcast_to(np.array([80.0, 40.0, 20.0] * 2, np.float32),
                              (128, 6)).copy(),
        "base3": (np.array([SB0, SB1, SB2], np.float32)[None, :]
                  + (j // NT_PAD).astype(np.float32)[:, None]
                  * np.array([S0, S1, S2], np.float32)[None, :]),
        "sidx": sidx.view(np.float32),
    }
    _CACHE["cblk"] = out
    return out


def _per_core_inputs(feat0, feat1, feat2, targets, core):
    b0 = core * BPC
    tpad = np.zeros((BPC, NT_PAD, 6), np.float32)
    tpad[:, :, 5] = 3.0  # pad rows match no layer
    tpad[:, :N_TGT, :] = targets[b0:b0 + BPC]
    tpad = tpad.reshape(NJ, 6)

    cb = _const_block()
    cpack = np.empty((128, CP_COLS), np.float32)
    cpack[:, CP_CLS:CP_LY + 1] = tpad[:, 0:6]
    cpack[:, CP_VALID:CP_VALID + 1] = cb["valid"]
    cpack[:, CP_IOTA3:CP_IOTA3 + 3] = cb["iota3"]
    cpack[:, CP_W6:CP_W6 + 6] = cb["w6"]
    cpack[:, CP_BASE3:CP_BASE3 + 3] = cb["base3"]
    cpack[:, CP_SIDX:CP_SIDX + 4] = cb["sidx"]

    # channels-last layout: feat[pos, c], pos = Sbase_l + b*S_l + fy*W + fx
    feat = np.concatenate([
        np.ascontiguousarray(
            f[b0:b0 + BPC].reshape(BPC, C, -1).transpose(0, 2, 1)
        ).reshape(-1, C)
        for f in (feat0, feat1, feat2)
    ])
    return {"feat": feat, "cpack": cpack}


def kernel(feat0, feat1, feat2, targets):
    nc, _ = _build()
    in_maps = [_per_core_inputs(feat0, feat1, feat2, targets, k)
               for k in range(N_CORES)]
    res = run_bass_kernel_spmd(nc, in_maps, core_ids=list(range(N_CORES)))
    parts = np.stack([r["out"][0:2, 0] for r in res.results])  # [8, 2]
    cls_sum = np.float32(parts[:, 0].sum(dtype=np.float32))
    box_sum = np.float32(parts[:, 1].sum(dtype=np.float32))
    total = np.float32(cls_sum + box_sum)
    return (total, cls_sum, box_sum)


# revision 19
# speedup vs baseline: 1.0063x; 1.0063x over previous
"""Trainium2 Bass kernel for DetectionLoss (focal cls + DFL box loss).

v3 strategy: DRAM-side sparse gather with latency-trimmed DMA chain.
-------------------------------------------------------------------
The loss reads the feature maps at only 50 target locations per image,
so per core the device gathers 128 padded targets x 144 channels
(576B runs) from a host-transposed channels-last copy of the feature
shard, instead of streaming the 9.7MB shard.  The per-rep critical path
is three serial DMA round trips (in -> gather -> out) plus the compute
between them; v3 trims each leg:

  1. Input DMA carries only 23 packed columns (targets + consts) on the
     SP/HWDGE queue (~2.8us incl. fixed DGE+sem latency).  iota rows are
     generated on-device by Pool while the DMA is in flight.
  2. Position math is vectorized over all 3 FPN layers at once on
     [128,6]/[128,3] tiles (floor emulated robustly: HW f32->i32 cast
     rounds-half-even, CoreSim truncates), ~11 DVE ops.
  3. One indirect SWDGE gather (128 x 576B descriptors) lands features
     in loss layout [target, channel].
  4. Both Exp and Ln come from act-func-set 6 (natural_log_exp_and_
     others), forced via a get_activation_tables patch, so the single
     1283ns table load runs at t=0 instead of mid-kernel.
  5. Loss math is spread across ACT/DVE/Pool with fused
     scalar_tensor_tensor(+accum) ops (tensor_tensor_reduce is rejected
     by this neuronxcc, so it is avoided).
  6. The 2-scalar result (valid-mask matmul in PSUM) leaves via a
     dma_scatter_add whose descriptors are PREPARED during the gather
     window and only TRIGGERED when the result tile is ready -- the
     output leg pays dge+transfer+sem (~1.6us) instead of the full
     ~2.75us HWDGE chain.  Distinct indices (0,1) make the "add" a
     plain store into the zero-initialized output buffer.

Host side does layout-only prep (transpose/concat/pad); all arithmetic
(indexing, softmax, focal, DFL, reduction) runs on device.  The 8 cores
are data-parallel over the batch; the host sums the 8 (cls, box) pairs.
"""

import numpy as np

import concourse.bass as bass
import concourse.hw_specs as hw_specs
import concourse.mybir as mybir
import concourse.tile as tile
from concourse.tile import add_dep_helper
from concourse import bacc
from concourse.bass_utils import run_bass_kernel_spmd

F32 = mybir.dt.float32
I32 = mybir.dt.int32
I16 = mybir.dt.int16
ALU = mybir.AluOpType
ACT = mybir.ActivationFunctionType
AX = mybir.AxisListType

OUT_MODE = "plain"   # "scatter" (prep+trigger) is broken on HW ucode: q0 wedges, q1 reads stale src; keep Pool dma_start

N_CORES = 8
B = 16
BPC = B // N_CORES          # images per core
N_TGT = 50
NT_PAD = 64                 # padded targets per image
NJ = BPC * NT_PAD           # 128 padded targets per core
N_CLS = 80
N_BINS = 16
C = 4 * N_BINS + N_CLS      # 144
S0, S1, S2 = 6400, 1600, 400
NPOS = BPC * (S0 + S1 + S2)            # 16800 spatial positions/core
SB0, SB1, SB2 = 0, BPC * S0, BPC * (S0 + S1)  # layer position bases

# packed-constant column layout (f32 columns)
CP_CLS = 0
CP_CX = 1
CP_CY = 2
CP_W = 3
CP_H = 4
CP_LY = 5
CP_VALID = 6
CP_IOTA3 = 7                # 0,1,2
CP_W6 = 10                  # 80,40,20,80,40,20
CP_BASE3 = 16               # SB_l + (j//64)*S_l
CP_IO80 = 19                # iota 0..79 (one-hot compare row)
CP_IO64 = 99                # iota 0..15 x4 (tent-weight row)
CP_COLS = 163


def _patch_act_tables():
    """Force every Exp/Ln activation onto act-func-set 6
    (natural_log_exp_and_others) so one t=0 table load covers both.
    Only the set-selection input to Bacc's load-insertion pass changes;
    emitted ids still index the real act_info.json."""
    if getattr(bacc, "_act_tables_patched", False):
        return
    orig = hw_specs.get_activation_tables
    both = {ACT.Exp, ACT.Ln}

    def patched(arch):
        tabs = orig(arch)
        out = {}
        for name, funcs in tabs.items():
            if name == "natural_log_exp_and_others":
                out[name] = set(funcs)
            else:
                out[name] = set(funcs) - both
        return out

    bacc.get_activation_tables = patched
    bacc._act_tables_patched = True


def _emit(nc, tc, io, pools, mode="full"):
    pw, pp = pools

    def ts(eng, dst, src, s1, s2, o1, o2=None):
        if o2 is None:
            eng.tensor_scalar(dst, src, s1, s2, o1)
        else:
            eng.tensor_scalar(dst, src, s1, s2, o1, o2)

    V, P, A = nc.vector, nc.gpsimd, nc.scalar

    # ---- t0: SP issues the input DMA (iota rows ride along in cpack:
    # gpsimd.iota is a Q7 software op, ~75us per call on real HW) ----
    cp = pw.tile([128, CP_COLS], F32, tag="cp")
    nc.sync.dma_start(cp[:], io["cpack"])

    cls_c = cp[:, CP_CLS:CP_CLS + 1]
    ly_c = cp[:, CP_LY:CP_LY + 1]
    valid = cp[:, CP_VALID:CP_VALID + 1]
    iota3 = cp[:, CP_IOTA3:CP_IOTA3 + 3]
    io80 = cp[:, CP_IO80:CP_IO80 + N_CLS]
    io64 = cp[:, CP_IO64:CP_IO64 + 64]
    w6 = cp[:, CP_W6:CP_W6 + 6]
    w3 = cp[:, CP_W6:CP_W6 + 3]
    base3 = cp[:, CP_BASE3:CP_BASE3 + 3]

    # ---- position index per target (vectorized over 3 layers) ----
    cxy_b = cp[:, CP_CX:CP_CX + 2].unsqueeze(2).to_broadcast([128, 2, 3])
    c6 = pw.tile([128, 2, 3], F32, tag="c6")
    V.tensor_tensor(c6[:], cxy_b, w6[:].rearrange("p (a b) -> p a b", b=3),
                    ALU.mult)
    r6 = pw.tile([128, 2, 3], I32, tag="r6")
    V.tensor_copy(r6[:], c6[:])
    f6 = pw.tile([128, 2, 3], F32, tag="f6")
    V.tensor_copy(f6[:], r6[:])
    a6 = pw.tile([128, 2, 3], F32, tag="a6")
    V.tensor_tensor(a6[:], f6[:], c6[:], ALU.is_gt)
    s6 = pw.tile([128, 2, 3], F32, tag="s6")
    V.tensor_tensor(s6[:], f6[:], a6[:], ALU.subtract)  # [fx3 | fy3]
    E = pw.tile([128, 3], F32, tag="E")
    V.tensor_tensor(E[:], iota3, ly_c.to_broadcast([128, 3]), ALU.is_equal)
    m3 = pw.tile([128, 3], F32, tag="m3")
    V.tensor_tensor(m3[:], s6[:, 1, :], w3, ALU.mult)
    V.tensor_tensor(m3[:], m3[:], s6[:, 0, :], ALU.add)
    V.tensor_tensor(m3[:], m3[:], base3, ALU.add)
    pj = pw.tile([128, 3], F32, tag="pj")
    posf = pw.tile([128, 1], F32, tag="posf")
    V.scalar_tensor_tensor(pj[:], m3[:], 1.0, E[:], ALU.mult, ALU.mult,
                           accum_out=posf[:])
    ix = pw.tile([128, 1], I32, tag="ix")
    ix_inst = V.tensor_copy(ix[:], posf[:])  # exact int; cast mode free

    # ---- gather target columns from HBM (128 x 576B), then prep out ----
    T = pw.tile([128, C], F32, tag="T")
    P.indirect_dma_start(
        out=T[:], out_offset=None, in_=io["feat"],
        in_offset=bass.IndirectOffsetOnAxis(ap=ix[:], axis=0))
    if mode == "gather":
        P.dma_start(io["out"], T[0:2, 0:1])
        return

    # ---- target-only precomputes (overlap the gather) ----
    # nosync edges on ix keep these off the critical pos chain: the Tile
    # list scheduler is greedy and would otherwise slot them into the
    # chain's semaphore-wait bubbles, displacing the gather by ~800ns.
    pre = []
    oh = pw.tile([128, N_CLS], F32, tag="oh")
    pre.append(V.tensor_tensor(oh[:], io80,
                               cls_c.to_broadcast([128, N_CLS]),
                               ALU.is_equal))
    Wt = pw.tile([128, 1], F32, tag="Wt")
    ew = pw.tile([128, 3], F32, tag="ew")
    pre.append(V.scalar_tensor_tensor(ew[:], E[:], 1.0, w3, ALU.mult,
                                      ALU.mult, accum_out=Wt[:]))
    hh = pw.tile([128, 1], F32, tag="hh")
    pre.append(V.tensor_scalar(hh[:], Wt[:], 0.5, None, ALU.mult))
    g1 = pw.tile([128, 1], F32, tag="g1")
    g2 = pw.tile([128, 1], F32, tag="g2")
    pre.append(V.tensor_tensor(g1[:], cp[:, CP_W:CP_W + 1], hh[:],
                               ALU.mult))
    pre.append(V.tensor_tensor(g2[:], cp[:, CP_H:CP_H + 1], hh[:],
                               ALU.mult))
    t4 = pw.tile([128, 4], F32, tag="t4")
    t4v = t4[:].rearrange("p (a b) -> p a b", b=2)
    pre.append(V.tensor_copy(t4v[:, :, 0:1],
                             g1[:].unsqueeze(2).to_broadcast([128, 2, 1])))
    pre.append(V.tensor_copy(t4v[:, :, 1:2],
                             g2[:].unsqueeze(2).to_broadcast([128, 2, 1])))
    pre.append(V.tensor_scalar(t4[:], t4[:],
                               float(N_BINS - 1 - 1e-06), None, ALU.min))
    # tent weights Wb[t,(a,k)] = relu(1 - |k - t4[t,a]|)
    Wb = pw.tile([128, 64], F32, tag="Wb")
    pre.append(V.tensor_tensor(
        Wb[:].rearrange("p (a b) -> p a b", b=N_BINS),
        io64.rearrange("p (a b) -> p a b", b=N_BINS),
        t4[:].unsqueeze(2).to_broadcast([128, 4, N_BINS]),
        ALU.subtract))
    wa = pw.tile([128, 64], F32, tag="wa")
    pre.append(V.tensor_scalar(wa[:], Wb[:], -1.0, 1.0, ALU.mult, ALU.add))
    pre.append(V.tensor_scalar(Wb[:], Wb[:], 1.0, None, ALU.add))
    pre.append(V.tensor_tensor(Wb[:], wa[:], Wb[:], ALU.min))
    pre.append(V.tensor_scalar(Wb[:], Wb[:], 0.0, None, ALU.max))
    for p_ in pre:
        add_dep_helper(p_.ins, ix_inst.ins, sync=False,
                       reason="precompute after pos chain")

    # ---- post-gather loss math ----
    d64 = T[:, 0:64]
    z80 = T[:, 64:C]
    lt = pw.tile([128, 2], F32, tag="lt")   # [sez, prod4]

    ed = pw.tile([128, 64], F32, tag="ed")
    A.activation(ed[:], d64, ACT.Exp)
    ez = pw.tile([128, N_CLS], F32, tag="ez")
    A.activation(ez[:], z80, ACT.Exp, accum_out=lt[:, 0:1])

    wdj = pw.tile([128, 64], F32, tag="wdj")
    wds = pw.tile([128, 1], F32, tag="wds")
    V.scalar_tensor_tensor(wdj[:], Wb[:], 1.0, d64, ALU.mult, ALU.mult,
                           accum_out=wds[:])
    zmj = pw.tile([128, N_CLS], F32, tag="zmj")
    zsel = pw.tile([128, 1], F32, tag="zsel")
    V.scalar_tensor_tensor(zmj[:], z80, 1.0, oh[:], ALU.mult, ALU.mult,
                           accum_out=zsel[:])
    se4 = pw.tile([128, 4], F32, tag="se4")
    V.tensor_reduce(se4[:], ed[:].rearrange("p (a b) -> p a b", b=N_BINS),
                    AX.X, ALU.add)
    pr2 = pw.tile([128, 2], F32, tag="pr2")
    V.tensor_tensor(pr2[:], se4[:, 0:2], se4[:, 2:4], ALU.mult)
    V.tensor_tensor(lt[:, 1:2], pr2[:, 0:1], pr2[:, 1:2], ALU.mult)

    emj = pw.tile([128, N_CLS], F32, tag="emj")
    esel = pw.tile([128, 1], F32, tag="esel")
    V.scalar_tensor_tensor(emj[:], ez[:], 1.0, oh[:], ALU.mult, ALU.mult,
                           accum_out=esel[:])

    ll = pw.tile([128, 2], F32, tag="ll")   # [lse, ln prod4]
    A.activation(ll[:], lt[:], ACT.Ln)

    rse = pw.tile([128, 1], F32, tag="rse")
    V.reciprocal(rse[:], lt[:, 0:1])
    pt = pw.tile([128, 1], F32, tag="pt")
    V.tensor_tensor(pt[:], esel[:], rse[:], ALU.mult)
    u1 = pw.tile([128, 1], F32, tag="u1")
    ts(V, u1[:], pt[:], -1.0, 1.0, ALU.mult, ALU.add)
    u2 = pw.tile([128, 1], F32, tag="u2")
    V.tensor_tensor(u2[:], u1[:], u1[:], ALU.mult)
    S = pw.tile([128, 2], F32, tag="S")
    cev = pw.tile([128, 1], F32, tag="cev")
    V.scalar_tensor_tensor(cev[:], ll[:, 0:1], zsel[:], valid,
                           ALU.subtract, ALU.mult)
    V.tensor_tensor(S[:, 0:1], u2[:], cev[:], ALU.mult)
    V.scalar_tensor_tensor(S[:, 1:2], ll[:, 1:2], wds[:], valid,
                           ALU.subtract, ALU.mult)

    # ---- reduce 128 per-target contributions to 2 scalars, send out ----
    # valid is already folded into both S columns (ones column would do;
    # reusing valid is free).  gpsimd.tensor_reduce(axis=C) is a Q7
    # software op (~150us on HW!) -- use the PE matmul instead.
    PS = pp.tile([2, 1], F32, tag="PS")
    nc.tensor.matmul(PS[:], S[:], valid, start=True, stop=True)
    osb = pw.tile([2, 1], F32, tag="osb")
    V.tensor_copy(osb[:], PS[:])
    P.dma_start(io["out"], osb[:])


_CACHE = {}


def _build(reps=1, mode="full"):
    key = f"nc{reps}_{mode}_{OUT_MODE}"
    if key in _CACHE:
        return _CACHE[key], _CACHE[key + "_names"]
    _patch_act_tables()
    nc = bacc.Bacc("TRN2", target_bir_lowering=False, debug=False,
                   enable_asserts=False, num_devices=N_CORES,
                   num_swdge_queues=2)
    io = {}
    io["feat"] = nc.dram_tensor("feat", [NPOS, C], F32,
                                kind="ExternalInput").ap()
    io["cpack"] = nc.dram_tensor("cpack", [128, CP_COLS], F32,
                                 kind="ExternalInput").ap()
    io["out"] = nc.dram_tensor("out", [2, 1], F32,
                               kind="ExternalOutput").ap()

    with tile.TileContext(nc) as tc:
        with tc.tile_pool(name="wk", bufs=1) as pw, \
             tc.tile_pool(name="ps", bufs=1, space="PSUM") as pp:
            for r in range(reps):
                if r:
                    tc.strict_bb_all_engine_barrier()
                _emit(nc, tc, io, (pw, pp), mode=mode)
    nc.compile()
    _CACHE[key] = nc
    _CACHE[key + "_names"] = list(io)
    return nc, list(io)


def _const_block():
    if "cblk" in _CACHE:
        return _CACHE["cblk"]
    j = np.arange(NJ)
    out = {
        "valid": ((j % NT_PAD) < N_TGT).astype(np.float32)[:, None],
        "iota3": np.broadcast_to(np.arange(3, dtype=np.float32),
                                 (128, 3)).copy(),
        "w6": np.broadcast_to(np.array([80.0, 40.0, 20.0] * 2, np.float32),
                              (128, 6)).copy(),
        "base3": (np.array([SB0, SB1, SB2], np.float32)[None, :]
                  + (j // NT_PAD).astype(np.float32)[:, None]
                  * np.array([S0, S1, S2], np.float32)[None, :]),
        "io80": np.broadcast_to(np.arange(N_CLS, dtype=np.float32),
                                (128, N_CLS)).copy(),
        "io64": np.broadcast_to(np.tile(np.arange(N_BINS, dtype=np.float32),
                                        4), (128, 64)).copy(),
    }
    _CACHE["cblk"] = out
    return out


def _per_core_inputs(feat0, feat1, feat2, targets, core):
    b0 = core * BPC
    tpad = np.zeros((BPC, NT_PAD, 6), np.float32)
    tpad[:, :, 5] = 3.0  # pad rows match no layer
    tpad[:, :N_TGT, :] = targets[b0:b0 + BPC]
    tpad = tpad.reshape(NJ, 6)

    cb = _const_block()
    cpack = np.empty((128, CP_COLS), np.float32)
    cpack[:, CP_CLS:CP_LY + 1] = tpad[:, 0:6]
    cpack[:, CP_VALID:CP_VALID + 1] = cb["valid"]
    cpack[:, CP_IOTA3:CP_IOTA3 + 3] = cb["iota3"]
    cpack[:, CP_W6:CP_W6 + 6] = cb["w6"]
    cpack[:, CP_BASE3:CP_BASE3 + 3] = cb["base3"]
    cpack[:, CP_IO80:CP_IO80 + N_CLS] = cb["io80"]
    cpack[:, CP_IO64:CP_IO64 + 64] = cb["io64"]

    # channels-last layout: feat[pos, c], pos = Sbase_l + b*S_l + fy*W + fx
    feat = np.concatenate([
        np.ascontiguousarray(
            f[b0:b0 + BPC].reshape(BPC, C, -1).transpose(0, 2, 1)
        ).reshape(-1, C)
        for f in (feat0, feat1, feat2)
    ])
    return {"feat": feat, "cpack": cpack}


def kernel(feat0, feat1, feat2, targets):
    nc, _ = _build()
    in_maps = [_per_core_inputs(feat0, feat1, feat2, targets, k)
               for k in range(N_CORES)]
    res = run_bass_kernel_spmd(nc, in_maps, core_ids=list(range(N_CORES)))
    parts = np.stack([r["out"][0:2, 0] for r in res.results])  # [8, 2]
    cls_sum = np.float32(parts[:, 0].sum(dtype=np.float32))
    box_sum = np.float32(parts[:, 1].sum(dtype=np.float32))
    total = np.float32(cls_sum + box_sum)
    return (total, cls_sum, box_sum)
